# revision 52
# baseline (speedup 1.0000x reference)
"""Causal self-attention kernel for 8 trn2 NeuronCores (Bass/Tile), v3.

Problem: B=4, T=2048, C=1024, H=16 heads, D=64. f32 in/out.
  qkv = x @ w_attn.T + b_attn ; causal softmax attention ; y @ w_proj.T + b_proj

Sharding: core i handles batch b=i//2, head-group g=i%2 (8 heads each).
Each core computes a partial projection output [T, C]; the host sums the
two head-group partials per batch and adds b_proj (exact in fp32).

v8 = v3 + PE-stall fixes + fp8 q/k projection (420us -> ~295us noisy,
~230us under clean conditions):
 - causal mask folded into the QK PSUM accumulation as one extra PE
   matmul per diagonal block (U1=I, U2 strict-lower so U1^T@U2 adds a
   constant negative offset above the diagonal; exp then sees ~-64 ->
   ~1e-28, which the table-based ACT exp handles, unlike huge
   out-of-domain args).
 - normalization: per head-pair, copy O + den rows PSUM->SBUF first
   (frees the ot PSUM pair fast; the old order queued 3.3us exact
   reciprocals ahead of the copies, stalling AV ~5.3us per pair and
   HAM-re-throttling the PE to 1.2GHz). Per chunk: gather 4096
   denominators to [1,4096], DMA-spread to [64,64] across partitions,
   ONE exact DVE reciprocal (~0.4us), bounce back, [0,64]-stride
   broadcast. Mid-chain hops ride the gpsimd DMA queue. (ACT ln/exp
   thrashes act tables; reciprocal_approx_fast is garbage on HW.)
 - q/k projection in fp8e4m3 with DoubleRow perf mode (K=256 per
   matmul, 2x ALU): w_qk scaled x64 into fp8 range, compensated in
   bqk (x64) and the exp scale (0.125/4096); v/proj stay bf16.
   rel err 9.9e-3 vs the 2e-2 gate.
 - proj fillers carry into the next chunk instead of force-flushing at
   the boundary (they wait on the normalization chain and blocked the
   in-order PE queue); y = O*(1/den) muls split across DVE and Pool.
"""
import sys
sys.path.insert(0, "/opt/trn_rl_repo")

from contextlib import ExitStack

import numpy as np

import concourse.bass as bass
import concourse.tile as tile
from concourse import bacc, mybir
from concourse.bass_utils import run_bass_kernel_spmd

F32 = mybir.dt.float32
BF16 = mybir.dt.bfloat16
FP8 = mybir.dt.float8e4
EXP = mybir.ActivationFunctionType.Exp
LN = mybir.ActivationFunctionType.Ln
N_CORES = 8
B, T, C = 4, 2048, 1024
H, D = 16, 64          # global heads
HL = 8                 # heads per core
CL = HL * D            # 512 local channels


def build_nc(reps: int = 1):
    """Build the SPMD Bass program (same on all cores)."""
    nc = bacc.Bacc("TRN2", target_bir_lowering=False, debug=False,
                   num_devices=N_CORES)
    xT_d = nc.dram_tensor("xT", [C, T], BF16, kind="ExternalInput").ap()
    xT8_d = nc.dram_tensor("xT8", [C, T], FP8, kind="ExternalInput").ap()
    wqk8T_d = nc.dram_tensor("wqk8T", [C, 2 * CL], FP8,
                             kind="ExternalInput").ap()
    wvT_d = nc.dram_tensor("wvT", [C, CL], BF16, kind="ExternalInput").ap()
    bqk_d = nc.dram_tensor("bqk", [2 * CL, 1], F32, kind="ExternalInput").ap()
    bv_d = nc.dram_tensor("bv", [128, CL], BF16, kind="ExternalInput").ap()
    wpT_d = nc.dram_tensor("wpT", [CL, C], BF16, kind="ExternalInput").ap()
    u1_d = nc.dram_tensor("u1", [128, 128], BF16, kind="ExternalInput").ap()
    u2_d = nc.dram_tensor("u2", [128, 128], BF16, kind="ExternalInput").ap()
    one_d = nc.dram_tensor("one", [128, 128], BF16, kind="ExternalInput").ap()
    out_d = nc.dram_tensor("out", [T, C], F32, kind="ExternalOutput").ap()

    xT_r = xT_d.rearrange("(c p) t -> p c t", p=128)
    xT8_r = xT8_d.rearrange("(c p) t -> p c t", p=128)
    total = reps * 4

    with tile.TileContext(nc) as tc, ExitStack() as top:
        top.enter_context(nc.allow_low_precision(
            reason="bf16 pipeline validated vs f32 reference: rel err ~3e-3"))
        persist = top.enter_context(tc.tile_pool(name="persist", bufs=1))
        # parity-double-buffered K^T / V_aug quarters
        # K^T: [part (2 heads of pair m), m, s-cols]
        kt_q = [[persist.tile([128, 4, 512], BF16, name=f"ktq{p}{q}",
                              tag=f"ktq{p}{q}") for q in range(4)]
                for p in range(2)]
        # V_aug: [s-part, s-block, 8 heads x (64 v | ones)]
        v_q = [[persist.tile([128, 4, 8 * 65], BF16, name=f"vq{p}{q}",
                             tag=f"vq{p}{q}") for q in range(4)]
               for p in range(2)]
        w8_sb = persist.tile([128, 8, 2 * CL], FP8)   # qk weights (x64, fp8)
        wv_sb = persist.tile([128, 8, CL], BF16)      # v weights
        wp_sb = persist.tile([128, 4, C], BF16)
        bqk_sb = persist.tile([128, 8], F32)
        bv_sb = persist.tile([128, CL], BF16)
        u1_sb = persist.tile([128, 128], BF16)
        u2_sb = persist.tile([128, 128], BF16)

        for ot in range(8):
            nc.sync.dma_start(out=bqk_sb[:, ot:ot + 1],
                              in_=bqk_d[ot * 128:(ot + 1) * 128, :])
        nc.sync.dma_start(out=bv_sb[:], in_=bv_d[:])
        nc.sync.dma_start(out=u1_sb[:], in_=u1_d[:])
        nc.sync.dma_start(out=u2_sb[:], in_=u2_d[:])
        for p in range(2):
            for q in range(4):
                vdst = v_q[p][q][:, :, :].rearrange(
                    "p k (h x) -> p k h x", x=65)[:, :, :, 64:65]
                nc.sync.dma_start(
                    out=vdst,
                    in_=one_d[:, q * 32:(q + 1) * 32].rearrange(
                        "p (k h) -> p k h", h=8)[:, :, :, None])
        nc.gpsimd.dma_start(
            out=w8_sb[:],
            in_=wqk8T_d.rearrange("(c p) o -> p c o", p=128))
        nc.gpsimd.dma_start(
            out=wv_sb[:],
            in_=wvT_d.rearrange("(c p) o -> p c o", p=128))
        nc.gpsimd.dma_start(
            out=wp_sb[:],
            in_=wpT_d.rearrange("(c p) o -> p c o", p=128))

        with tc.tile_pool(name="xtp", bufs=2) as xtp, \
             tc.tile_pool(name="qtp", bufs=2) as qtp, \
             tc.tile_pool(name="ytp", bufs=2) as ytp, \
             tc.tile_pool(name="ptp", bufs=2) as ptp, \
             tc.tile_pool(name="osb", bufs=2) as osb, \
             tc.tile_pool(name="oc2", bufs=8) as oc2, \
             tc.tile_pool(name="ph2", bufs=1) as ph2, \
             tc.tile_pool(name="ps1", bufs=2, space="PSUM") as ps1, \
             tc.tile_pool(name="stp", bufs=2, space="PSUM") as stp, \
             tc.tile_pool(name="otp", bufs=1, space="PSUM") as otp, \
             tc.tile_pool(name="drp", bufs=2, space="DRAM") as drp:

            stores = {}
            yts = {}
            pending_b = []

            def load_thunk(gc):
                def load():
                    xt = xtp.tile([128, 8, 512], BF16, tag="xt")
                    x8t = xtp.tile([128, 8, 512], FP8, tag="x8t")
                    stores[gc] = {"xt": xt, "x8t": x8t}
                    t0 = (gc % 4) * 512
                    nc.sync.dma_start(out=xt[:], in_=xT_r[:, :, t0:t0 + 512])
                    nc.sync.dma_start(out=x8t[:],
                                      in_=xT8_r[:, :, t0:t0 + 512])
                return load

            def qkv_thunks(gc):
                par = (gc // 4) % 2
                q = gc % 4
                ths = []

                def prep():
                    stores[gc]["qt"] = qtp.tile([128, 4, 512], BF16,
                                                name="qt", tag="qt")
                ths.append(prep)
                for ot in range(8):
                    def g(ot=ot):
                        x8t = stores[gc]["x8t"]
                        ps = ps1.tile([128, 512], F32, tag="ps1")
                        # fp8 DoubleRow: 2 K-tiles (256 contraction) per
                        # matmul at 2x ALU rate; q,k carry a x64 scale
                        # (folded into bqk and the exp scale)
                        for c2 in range(4):
                            nc.tensor.matmul(
                                ps[:],
                                w8_sb[:, 2 * c2:2 * c2 + 2,
                                      ot * 128:(ot + 1) * 128],
                                x8t[:, 2 * c2:2 * c2 + 2, :],
                                start=(c2 == 0), stop=(c2 == 3),
                                perf_mode=mybir.MatmulPerfMode.DoubleRow)
                        if ot < 4:
                            dst = stores[gc]["qt"][:, ot, :]
                        else:
                            dst = kt_q[par][q][:, ot - 4, :]
                        nc.vector.tensor_scalar_add(dst, ps[:],
                                                    bqk_sb[:, ot:ot + 1])
                    ths.append(g)
                for vt in range(4):
                    def gv(vt=vt):
                        xt = stores[gc]["xt"]
                        ps = ps1.tile([128, 512], F32, tag="ps1")
                        for c in range(8):
                            nc.tensor.matmul(
                                ps[:],
                                xt[:, c, vt * 128:(vt + 1) * 128],
                                wv_sb[:, c, :],
                                start=(c == 0), stop=(c == 7))
                        vk = v_q[par][q][:, vt, :].rearrange(
                            "p (h x) -> p h x", x=65)[:, :, 0:64]
                        nc.vector.tensor_add(
                            vk, ps[:].rearrange("p (h x) -> p h x", x=64),
                            bv_sb[:].rearrange("p (h x) -> p h x", x=64))
                    ths.append(gv)
                return ths

            def proj_thunks(gc):
                tq = gc % 4
                yt = yts[gc]
                ths = []
                for tb4 in range(4):
                    def g(tb4=tb4, yt=yt):
                        ob = osb.tile([128, 1024], F32, tag="o")
                        for o2 in range(2):
                            ps = ps1.tile([128, 512], F32, tag="ps1")
                            for hc in range(4):
                                nc.tensor.matmul(
                                    ps[:],
                                    yt[:, hc, tb4 * 128:(tb4 + 1) * 128],
                                    wp_sb[:, hc, o2 * 512:(o2 + 1) * 512],
                                    start=(hc == 0), stop=(hc == 3))
                            nc.vector.tensor_copy(
                                ob[:, o2 * 512:(o2 + 1) * 512], ps[:])
                        nc.sync.dma_start(
                            out=out_d[tq * 512 + tb4 * 128:
                                      tq * 512 + (tb4 + 1) * 128, :],
                            in_=ob[:])
                    ths.append(g)
                return ths

            load_thunk(0)()
            load_thunk(1)()
            for th in qkv_thunks(0):
                th()
            for gc in range(total):
                tq = gc % 4
                par = (gc // 4) % 2
                tcs = tq * 512
                kmaxc = (tcs + 512) // 128
                qt = stores[gc]["qt"]
                yt_c = ytp.tile([128, 4, 512], BF16, tag="yt")
                yts[gc] = yt_c
                fillers = []
                if gc + 2 < total:
                    fillers.append(load_thunk(gc + 2))
                # qkv thunks first: proj(gc-1) reads yt of ALL 4 head
                # pairs of gc-1, and the last pair's yt hangs off the
                # ~8us normalization DMA chain -- an early proj filler
                # would block the in-order PE queue on it.
                a = qkv_thunks(gc + 1) if gc + 1 < total else []
                b = proj_thunks(gc - 1) if gc > 0 else []
                fillers.extend(a)
                fillers.extend(pending_b)
                fillers.extend(b)
                slots = 4 * kmaxc
                emitted = 0
                slot = 0
                ocs = {}
                dnt = ph2.tile([1, 4 * 1024], F32, tag="dnt")
                for m in range(4):
                    # head pair A=2m (partitions 0:64), B=2m+1 (64:128)
                    ot_A = otp.tile([65, 512], F32, tag="otA")
                    ot_B = otp.tile([65, 512], F32, tag="otB")

                    def emit_ot(k, pt):
                        # software-pipelined: consumes pt of iteration k
                        t_lo = 128 * k
                        lo = max(tcs, t_lo)
                        kq, kk = k // 4, k % 4
                        for g, ot_g in ((0, ot_A), (1, ot_B)):
                            nc.tensor.matmul(
                                ot_g[0:65, lo - tcs:512],
                                v_q[par][kq][:, kk, (2 * m + g) * 65:
                                             (2 * m + g) * 65 + 65],
                                pt[:, g * 512 + lo - tcs:g * 512 + 512],
                                start=(k == 0), stop=(k == kmaxc - 1))

                    prev = None
                    for k in range(kmaxc):
                        t_lo = 128 * k
                        lo = max(tcs, t_lo)
                        kq, kk = k // 4, k % 4
                        st = stp.tile([128, 1024], F32, tag="st")
                        diag = tcs <= t_lo
                        for g, r0 in ((0, 0), (1, 64)):
                            nc.tensor.matmul(
                                st[:, g * 512 + lo - tcs:g * 512 + 512],
                                kt_q[par][kq][r0:r0 + 64, m,
                                              kk * 128:(kk + 1) * 128],
                                qt[r0:r0 + 64, m, lo - tcs:512],
                                start=True, stop=not diag,
                                skip_group_check=diag)
                        if diag:
                            # add exactly -512 above the diagonal of the
                            # diagonal block on the PE (U1=I, U2=-512*[j>c]
                            # so U1^T@U2 = -512*[key>q]). exp then sees
                            # ~-64, a legit f32 arg (~1e-28) -- the ACT
                            # exp TABLE mishandles huge out-of-domain args
                            # on HW, so the offset must stay moderate.
                            dc = t_lo - tcs
                            for g in (0, 1):
                                nc.tensor.matmul(
                                    st[:, g * 512 + dc:g * 512 + dc + 128],
                                    u1_sb[:], u2_sb[:],
                                    start=False, stop=True,
                                    skip_group_check=True)
                        pt = ptp.tile([128, 1024], BF16, tag="pt")
                        st3 = st[:].rearrange("p (g x) -> p g x", g=2)
                        pt3 = pt[:].rearrange("p (g x) -> p g x", g=2)
                        nc.scalar.activation(
                            pt3[:, :, lo - tcs:512],
                            st3[:, :, lo - tcs:512],
                            EXP, scale=0.125 / 4096.0)
                        if prev is not None:
                            emit_ot(prev[0], prev[1])
                        prev = (k, pt)
                        slot += 1
                        want = len(fillers) * slot // slots
                        while emitted < want:
                            fillers[emitted]()
                            emitted += 1
                    emit_ot(prev[0], prev[1])
                    # copy O (and the den rows into the per-chunk dnt
                    # gather tile) to SBUF FIRST -- frees the PSUM pair
                    # fast. Normalization is batched once per chunk.
                    oc = oc2.tile([64, 2, 512], F32, tag="oc")
                    ocs[m] = oc
                    nc.vector.tensor_copy(oc[:, 0, :], ot_A[0:64, :])
                    nc.vector.tensor_copy(
                        dnt[0:1, m * 1024:m * 1024 + 512], ot_A[64:65, :])
                    nc.vector.tensor_copy(oc[:, 1, :], ot_B[0:64, :])
                    nc.vector.tensor_copy(
                        dnt[0:1, m * 1024 + 512:m * 1024 + 1024],
                        ot_B[64:65, :])
                # batched normalization, once per chunk: bounce the 4096
                # denominators to DRAM, read back spread over 64
                # partitions, exact DVE reciprocal (free-size cost: 64
                # elems ~0.4us), bounce back, partition-broadcast. All
                # mid-chain hops ride the gpsimd DMA queue so the sync
                # queue (xt loads / out stores) never blocks on this.
                # (ACT ln/exp recip thrashes activation tables; DVE
                # [1,N] exact recip costs 6.5cyc/elem;
                # reciprocal_approx_fast returns garbage on HW.)
                dbo = drp.tile([1, 4096], F32, tag="dbo")
                nc.sync.dma_start(out=dbo[:], in_=dnt[:])
                dsp = ph2.tile([64, 64], F32, tag="dsp")
                nc.gpsimd.dma_start(
                    out=dsp[:],
                    in_=dbo[:].rearrange("o (a b) -> (o a) b", a=64))
                rcs = ph2.tile([64, 64], F32, tag="rcs")
                nc.vector.reciprocal(rcs[:], dsp[:])
                dbo2 = drp.tile([1, 4096], F32, tag="dbo2")
                nc.gpsimd.dma_start(
                    out=dbo2[:].rearrange("o (a b) -> (o a) b", a=64),
                    in_=rcs[:])
                rep_t = ph2.tile([64, 4, 1024], F32, tag="rep")
                dap = dbo2[0:1, :]
                bc = bass.AP(tensor=dap.tensor, offset=dap.offset,
                             ap=[[0, 64], [1, 4096]])
                nc.gpsimd.dma_start(
                    out=rep_t[:].rearrange("p m x -> p (m x)"), in_=bc)
                # all muls on Pool: they hang off the ~9.6us broadcast
                # DMA, and on the DVE FIFO they'd delay the next chunk's
                # PSUM-freeing copies; their own latency is harmless now
                # that proj fillers defer into the next chunk. oc2
                # bufs=8 breaks the cross-chunk WAR edge on oc reuse.
                for m in range(4):
                    for g in (0, 1):
                        eng = nc.gpsimd
                        eng.tensor_mul(
                            yt_c[g * 64:g * 64 + 64, m, :],
                            ocs[m][:, g, :],
                            rep_t[:, m, g * 512:g * 512 + 512])
                n_required = len(fillers) - len(b)
                while emitted < n_required:
                    fillers[emitted]()
                    emitted += 1
                pending_b = fillers[emitted:]
                if gc - 2 >= 0:
                    del yts[gc - 2]
                    del stores[gc - 1]
            for th in pending_b:
                th()
            for th in proj_thunks(total - 1):
                th()
    nc.compile()
    return nc


def make_in_maps(x, w_attn, b_attn, w_proj):
    bf = mybir.dt.np(BF16)
    # PE causal mask: (U1^T @ U2)[key,q] = -512*4096 for key>q else 0
    # exactly (st carries a x4096 scale from the fp8 x64 q/k scaling;
    # after the exp scale 0.125/4096 the masked arg is -64 -> ~1e-28)
    u1 = np.eye(128, dtype=np.float32).astype(bf)
    u2 = (-float(2 ** 21) *
          np.tril(np.ones((128, 128), dtype=np.float32), -1)).astype(bf)
    in_maps = []
    f8 = mybir.dt.np(FP8)
    xTs = [np.ascontiguousarray(x[b].T).astype(bf) for b in range(B)]
    xT8s = [np.ascontiguousarray(x[b].T).astype(f8) for b in range(B)]
    ones = np.ones((128, 128), dtype=np.float32).astype(bf)
    for i in range(N_CORES):
        b, g = i // 2, i % 2
        sl = slice(CL * g, CL * g + CL)
        wq = w_attn[0 * C:1 * C][sl.start:sl.stop]
        wk = w_attn[1 * C:2 * C][sl.start:sl.stop]
        wv = w_attn[2 * C:3 * C][sl.start:sl.stop]
        in_maps.append({
            "xT": xTs[b],
            "xT8": xT8s[b],
            "wqk8T": np.ascontiguousarray(
                64.0 * np.concatenate([wq, wk], 0).T).astype(f8),
            "wvT": np.ascontiguousarray(wv.T).astype(bf),
            "bqk": (64.0 * np.concatenate(
                [b_attn[0 * C:1 * C][sl.start:sl.stop],
                 b_attn[1 * C:2 * C][sl.start:sl.stop]]).reshape(2 * CL, 1))
                .astype(np.float32),
            "bv": np.broadcast_to(b_attn[2 * C:3 * C][sl.start:sl.stop],
                                  (128, CL)).astype(bf).copy(),
            "wpT": np.ascontiguousarray(w_proj[:, sl.start:sl.stop].T)
                .astype(bf),
            "u1": u1,
            "u2": u2,
            "one": ones,
        })
    return in_maps


_NC_CACHE = {}


def kernel(x, w_attn, b_attn, w_proj, b_proj):
    x = np.asarray(x, dtype=np.float32)
    w_attn = np.asarray(w_attn, dtype=np.float32)
    b_attn = np.asarray(b_attn, dtype=np.float32)
    w_proj = np.asarray(w_proj, dtype=np.float32)
    b_proj = np.asarray(b_proj, dtype=np.float32)

    if "nc" not in _NC_CACHE:
        _NC_CACHE["nc"] = build_nc()
    nc = _NC_CACHE["nc"]
    in_maps = make_in_maps(x, w_attn, b_attn, w_proj)
    res = run_bass_kernel_spmd(nc, in_maps, list(range(N_CORES))).results
    out = np.empty((B, T, C), dtype=np.float32)
    for b in range(B):
        out[b] = res[2 * b]["out"] + res[2 * b + 1]["out"] + b_proj
    return out

